# revision 30
# baseline (speedup 1.0000x reference)
"""Trainium2 Bass kernel for nn_AttentionPool_v1 (topk_masking).

Reference computation (N=16, C=64, W=512, H=512, RED=64, OUT_W=128):
    pooled = max(x, axes=(C, H))                  # [N, W]
    h      = pooled @ w1.T + b1                   # [N, RED]
    h      = BN1d(h, batch stats) -> relu         # [N, RED]
    att    = softmax(h @ w2.T + b2, axis=1)       # [N, W]
    idx    = sort(top_k(att, OUT_W).indices)      # [N, OUT_W]
    out    = x[n, :, idx[n], :]                   # [N, C, OUT_W, H]

Sharding: data-parallel over batch N across 8 cores (2 samples/core).
BatchNorm batch statistics are made exact via ONE tiny AllReduce of
[sum(h); sum(h^2)] packed into a [128, 1] payload (h is duplicated into
partitions 64..127 by loading w1/b1 twice), with var = E[h^2] - mu^2.

Device algorithm notes:
  * softmax is monotonic per-row => top-k on the logits z directly.
  * top-k via ranking: rank[i] = #{j: z[j] > z[i]} + #{j<i: z[j]==z[i]};
    keep = rank < OUT_W selects exactly the top-k set with jax.lax.top_k
    tie-breaking; output order is ascending index == jnp.sort(idx).
    The two rank sums are computed with fused scalar_tensor_tensor ops
    (compare + mask-multiply + accumulate in one instruction), reading
    the replicated z row straight out of PSUM.
  * compaction: one triu matmul gives within-chunk prefix sums, one
    ones matmul gives per-chunk totals; one-hot(pos) matmul iota gives
    the sorted idx values; indirect DMA row-gather moves the rows.
  * x is fed host-transposed as [n, w, c, h] so the (C,H) max-pool
    reads contiguous per-partition runs and the gather fetches
    contiguous (c, h) row slices with a single [128, 1] index column
    (multi-column indirect offsets hang the hardware DGE).
  * phase-1 x loads alternate between the two HWDGE rings (sync/scalar
    engines); constants load via the SWDGE ring to keep the x rings
    clean; phase-3 output writes also alternate the HWDGE rings.
"""

import numpy as np

import concourse.bacc as bacc
import concourse.bass as bass
import concourse.mybir as mybir
import concourse.tile as tile
from concourse.bass import IndirectOffsetOnAxis
from concourse.bass_utils import run_bass_kernel_spmd

F32 = mybir.dt.float32
I32 = mybir.dt.int32
ALU = mybir.AluOpType
AX = mybir.AxisListType
ACTF = mybir.ActivationFunctionType

# full-problem config
N_FULL, C_FULL, W_FULL, H_FULL = 16, 64, 512, 512
RED_FULL, OUTW_FULL = 64, 128
NCORES_FULL = 8
BN_EPS = 1e-5


class Cfg:
    def __init__(self, ncores=NCORES_FULL, n_loc=N_FULL // NCORES_FULL, c=C_FULL,
                 w=W_FULL, h=H_FULL, red=RED_FULL, out_w=OUTW_FULL,
                 chains=4, gw=8, cpt=16, gbufs=8, use_rdma=False):
        self.use_rdma = use_rdma          # remote-DMA batch-stats exchange
        assert w % 128 == 0
        self.ncores, self.n_loc, self.c, self.w, self.h = ncores, n_loc, c, w, h
        self.red, self.out_w = red, out_w
        self.wch = w // 128
        self.chains = chains              # x staging buffers (phase 1)
        self.gw = gw                      # channels per indirect gather
        self.gbufs = gbufs                # gather staging buffers (phase 3)
        self.cpt = min(cpt, c)            # channels per phase-1 tile
        assert c % self.cpt == 0 and c % gw == 0
        assert out_w <= 128 and red <= 128 and 2 * red <= 128


def kernel_body(tc, out_ap, ins, cfg: Cfg):
    """Emit the kernel IR. `ins` is a dict name -> DRAM AP (see host_inputs).

    Returns a list of (instruction, semaphore, threshold) waits to attach
    AFTER tile scheduling (cross-core arrivals the scheduler cannot model).
    """
    nc = tc.nc
    post_hooks = []
    n_loc, c, w, h, red, out_w = (cfg.n_loc, cfg.c, cfg.w, cfg.h, cfg.red,
                                  cfg.out_w)
    wch, gw = cfg.wch, cfg.gw
    xs = ins["xs"]
    group = [list(range(cfg.ncores))]
    nbatch = float(cfg.ncores * n_loc)

    # constants staged via the SWDGE ring (keeps HWDGE rings free for x)
    constp = tc.alloc_tile_pool(name="const", bufs=1)

    def cload(shape, src, name):
        t = constp.tile(shape, F32, name=name)
        nc.gpsimd.dma_start(t[:], ins[src])
        return t

    w1t_sb = cload([128, wch * 2 * red], "w1t2", "w1t")   # duplicated columns
    w2t_sb = cload([red, w], "w2t", "w2t")
    b1_sb = cload([128, 1], "b1d", "b1d")                 # duplicated rows
    gm_sb = cload([red, 1], "gmc", "gmc")
    bt_sb = cload([red, 1], "btc", "btc")
    b2_sb = cload([128, wch], "b2t", "b2t")
    idn_sb = cload([128, 128], "idn", "idn")
    ones_sb = cload([128, 128], "ones", "ones")
    triu_sb = cload([128, 128], "triu", "triu")
    irow_sb = cload([128, out_w], "irow", "irow")
    icol_sb = cload([128, wch], "icol", "icol")
    trim_sb = cload([128, wch * w], "trim", "trim")
    msk_sb = cload([128, 1], "mskhi", "mskhi")            # 1 on partitions>=red
    inv_sb = cload([128, 1], "msklo", "msklo")            # 1 on partitions<red
    eps_sb = cload([128, 1], "epsc", "epsc")              # BN_EPS everywhere

    # touch the Sqrt activation early: the first ACTIVATE on a fresh table
    # costs a ~1.3us ACT_TABLE_LOAD, which otherwise lands on the
    # post-collective critical path
    warmp = tc.alloc_tile_pool(name="warm", bufs=1)
    sq_warm = warmp.tile([64, 1], F32)
    nc.scalar.sqrt(sq_warm[:], eps_sb[0:64, :])
    warmp.release()

    mainp = tc.alloc_tile_pool(name="main", bufs=1)
    dramp = tc.alloc_tile_pool(name="dram", bufs=1, space="DRAM")
    psum_small = tc.alloc_tile_pool(name="ps_small", bufs=2, space="PSUM")
    psum_z = tc.alloc_tile_pool(name="ps_z", bufs=2, space="PSUM")

    hw_engs = [nc.sync, nc.scalar]        # the two HWDGE rings

    if cfg.use_rdma:
        # Batch-stats exchange via direct cross-core SBUF writes instead of
        # the ncfw collective.  Core r sends its [s1|s2] column pair to core
        # (r XOR d) landing at column pair 2d; sums over the landing tile are
        # permutation-invariant, so no rank-dependent addressing is needed.
        # Descriptors are prepared here (SWDGE ring is idle during phase 1)
        # and fired by one trigger_dma once the stats are written.
        stats2 = mainp.tile([128, 2], F32, name="stats2")
        land = mainp.tile([128, 2 * cfg.ncores], F32, name="land")
        nc.gpsimd.memset(stats2[:], 0.0)
        rd_rsem = nc.alloc_semaphore("rdma_stats_rsem")
        rd_lsem = nc.alloc_semaphore("rdma_stats_lsem")
        rd_ssem = nc.alloc_semaphore("rdma_stats_ssem")
        for dist in range(1, cfg.ncores):
            rdests = [None] * cfg.ncores
            rdests[dist] = (0, dist)
            nc.gpsimd.remote_dma_broadcast(
                out_ap=land[:, 2 * dist:2 * dist + 2],
                in_ap=stats2[:, 0:2],
                remote_sem=rd_rsem, local_sem=rd_lsem, rdests=rdests)

    # ---------------- phase 1: pooled[w] = max over (c, h) --------------
    # xs layout is [n, w, c, h] (host-transposed), so each w-partition
    # reads a contiguous (c, h) run.
    cpt = cfg.cpt                     # channels per tile
    nct = c // cpt
    pooledT = mainp.tile([128, wch * n_loc], F32)  # [p, k*n_loc+n]
    with tc.tile_pool(name="xp", bufs=cfg.chains) as xp, \
            tc.tile_pool(name="cmp", bufs=1) as cm_pool:
        colmax = []
        for n in range(n_loc):
            cm = cm_pool.tile([128, wch * nct], F32, name=f"colmax{n}",
                              tag=f"colmax{n}")
            colmax.append(cm)
        di = 0
        for n in range(n_loc):
            for k in range(wch):
                for t in range(nct):
                    xt = xp.tile([128, cpt * h], F32, tag="xt",
                                 name=f"xt{n}_{k}_{t}")
                    src = xs[n, k * 128:(k + 1) * 128, t * cpt:(t + 1) * cpt, :]
                    hw_engs[di % 2].dma_start(
                        out=xt[:].rearrange("p (cc hh) -> p cc hh", cc=cpt),
                        in_=src)
                    di += 1
                    nc.vector.reduce_max(
                        out=colmax[n][:, k * nct + t: k * nct + t + 1],
                        in_=xt[:], axis=AX.X)
        for n in range(n_loc):
            for k in range(wch):
                nc.vector.reduce_max(
                    out=pooledT[:, k * n_loc + n: k * n_loc + n + 1],
                    in_=colmax[n][:, k * nct:(k + 1) * nct], axis=AX.X)

    # ---------------- phase 2: MLP + BN + ranking -----------------------
    # hT2[p, nl]: partitions 0..red-1 and red..2*red-1 both hold h (the
    # duplicated w1/b1 make the matmul emit two copies), so one [128, 1]
    # reduction carries both sum(h) and sum(h^2) into a single AllReduce.
    hT_ps = psum_small.tile([2 * red, n_loc], F32, tag="mm")
    for k in range(wch):
        nc.tensor.matmul(out=hT_ps[:],
                         lhsT=w1t_sb[:, k * 2 * red:(k + 1) * 2 * red],
                         rhs=pooledT[:, k * n_loc:(k + 1) * n_loc],
                         start=(k == 0), stop=(k == wch - 1))
    hT2 = mainp.tile([2 * red, n_loc], F32)
    nc.vector.tensor_scalar_add(hT2[:], hT_ps[:], b1_sb[:, :1])

    if cfg.use_rdma:
        # stats2 = [sum_n h | sum_n h^2], both on partitions 0..red-1
        nc.vector.reduce_sum(out=stats2[0:red, 0:1], in_=hT2[0:red, :],
                             axis=AX.X)
        sq_scr = mainp.tile([red, n_loc], F32, name="sq_scr")
        nc.vector.scalar_tensor_tensor(
            out=sq_scr[:], in0=hT2[0:red, :], scalar=1.0, in1=hT2[0:red, :],
            op0=ALU.mult, op1=ALU.mult, accum_out=stats2[0:red, 1:2])
        nc.vector.tensor_copy(land[:, 0:2], stats2[:, 0:2])
        nc.gpsimd.trigger_dma(count=None).then_inc(rd_ssem, 1)
        # Arrival guard: waits on rd_ssem, incremented by the trigger
        # (which the scheduler's sim models and which transitively depends
        # on the stats being written), so the scheduler anchors the guard
        # late instead of hoisting a dep-less wait to the queue front.
        # The wait for the 7 peers' arrivals is attached post-scheduling
        # (the single-core sim cannot model remote semaphore increments);
        # build_nc asserts the guard still precedes the reduces in the
        # final program order.
        guard = nc.vector.wait_ge(rd_ssem, 1)
        land_v = land[:].rearrange("p (d s) -> p s d", s=2)
        allsum = mainp.tile([128, 2], F32, name="allsum")
        red1 = nc.vector.reduce_sum(out=allsum[:, 0:1], in_=land_v[:, 0:1, :],
                                    axis=AX.X)
        red2 = nc.vector.reduce_sum(out=allsum[:, 1:2], in_=land_v[:, 1:2, :],
                                    axis=AX.X)
        post_hooks.append((guard, rd_rsem, 2 * (cfg.ncores - 1), [red1, red2]))
        musum = allsum[0:red, 0:1]
        s2sum = allsum[0:red, 1:2]
    else:
        # g = h*mskhi + msklo  (1 on lower half, h on upper half)
        gsel = mainp.tile([2 * red, n_loc], F32)
        nc.vector.scalar_tensor_tensor(
            out=gsel[:], in0=hT2[:], scalar=msk_sb[:, :1],
            in1=inv_sb[:, :1].to_broadcast([2 * red, n_loc]),
            op0=ALU.mult, op1=ALU.add)
        # stats[p] = sum_n h*g  ->  lower half sum(h), upper half sum(h^2)
        scr_st = mainp.tile([2 * red, n_loc], F32)
        stats = mainp.tile([2 * red, 1], F32)
        nc.vector.scalar_tensor_tensor(
            out=scr_st[:], in0=hT2[:], scalar=1.0, in1=gsel[:],
            op0=ALU.mult, op1=ALU.mult, accum_out=stats[:])

        # two concurrent 64-element AllReduces (a single 128-element payload
        # trips a ~7x slower ncfw path); both inputs are ready together since
        # the variance is uncentered.  SWDGE staging keeps the input DMA off
        # the busy HWDGE completion-semaphore lanes.
        cc1_i = dramp.tile([red, 1], F32)
        cc1_o = dramp.tile([red, 1], F32)
        cc2_i = dramp.tile([red, 1], F32)
        cc2_o = dramp.tile([red, 1], F32)
        nc.gpsimd.dma_start(cc1_i[:], stats[0:red, :])
        nc.gpsimd.dma_start(cc2_i[:], stats[red:2 * red, :])
        nc.gpsimd.collective_compute("AllReduce", ALU.add,
                                     replica_groups=group,
                                     ins=[cc1_i.opt()], outs=[cc1_o.opt()])
        nc.gpsimd.collective_compute("AllReduce", ALU.add,
                                     replica_groups=group,
                                     ins=[cc2_i.opt()], outs=[cc2_o.opt()])
        musum = mainp.tile([red, 1], F32)
        s2sum = mainp.tile([red, 1], F32)
        nc.sync.dma_start(musum[:], cc1_o[:])
        nc.scalar.dma_start(s2sum[:], cc2_o[:])

    # rstd = 1/sqrt(s2sum/nb - mu^2 + eps);  hr = relu(gamma*rstd*(h-mu)+beta)
    mu = mainp.tile([red, 1], F32)
    nc.vector.tensor_scalar_mul(mu[:], musum[:], 1.0 / nbatch)
    nmu = mainp.tile([red, 1], F32)
    nc.scalar.mul(nmu[:], mu[:], -1.0)
    emm = mainp.tile([red, 1], F32)                      # eps - mu^2
    nc.vector.scalar_tensor_tensor(
        out=emm[:], in0=mu[:], scalar=nmu[:, :1],
        in1=eps_sb[0:red, :1].to_broadcast([red, 1]),
        op0=ALU.mult, op1=ALU.add)
    sd = mainp.tile([red, 1], F32)
    nc.scalar.activation(sd[:], s2sum[:], ACTF.Sqrt,
                         bias=emm[:, :1], scale=1.0 / nbatch)
    rstd = mainp.tile([red, 1], F32)
    nc.vector.reciprocal(rstd[:], sd[:])
    gs = mainp.tile([red, 1], F32)
    nc.vector.tensor_mul(gs[:], gm_sb[:], rstd[:])
    centered = mainp.tile([red, n_loc], F32)
    nc.vector.tensor_scalar_sub(centered[:], hT2[0:red, :], mu[:, :1])
    hr = mainp.tile([red, n_loc], F32)
    nc.vector.scalar_tensor_tensor(
        out=hr[:], in0=centered[:], scalar=gs[:, :1],
        in1=bt_sb[:, :1].to_broadcast([red, n_loc]),
        op0=ALU.mult, op1=ALU.add)
    nc.vector.tensor_scalar_max(hr[:], hr[:], 0.0)

    # zT[wq, nl] = w2 @ hr + b2   (logits, transposed)
    zT = mainp.tile([128, wch * n_loc], F32)  # col = q*n_loc + nl
    for q in range(wch):
        z_ps = psum_small.tile([128, n_loc], F32, tag="mm")
        nc.tensor.matmul(out=z_ps[:], lhsT=w2t_sb[:, q * 128:(q + 1) * 128],
                         rhs=hr[:], start=True, stop=True)
        nc.vector.tensor_scalar_add(zT[:, q * n_loc:(q + 1) * n_loc], z_ps[:],
                                    b2_sb[:, q:q + 1])

    # per-sample: ranking -> keep -> pos -> idx -> row gather.  Sample 0's
    # gather DMAs start while sample 1's ranking still runs.
    rank0 = mainp.tile([128, wch * n_loc], F32)
    rankt = mainp.tile([128, wch * n_loc], F32)
    rankf = mainp.tile([128, wch * n_loc], F32)
    keep = mainp.tile([128, wch * n_loc], F32)
    x_rows = xs.rearrange("n w c h -> (n w) (c h)")
    scrp = tc.alloc_tile_pool(name="scr", bufs=2 * wch)
    ohp = tc.alloc_tile_pool(name="ohp", bufs=wch * n_loc)
    gp = tc.alloc_tile_pool(name="gp", bufs=cfg.gbufs)
    ones_bc = ones_sb[:, 0:1].to_broadcast([128, w])

    for nl in range(n_loc):
        # replicated logits row, straight in PSUM: zrow[p, j] = z[j]
        zrow = psum_z.tile([128, w], F32, tag="zrow", name=f"zrow{nl}")
        for q in range(wch):
            col = q * n_loc + nl
            nc.tensor.transpose(
                out=zrow[:, q * 128:(q + 1) * 128],
                in_=zT[:, col:col + 1].to_broadcast([128, 128]),
                identity=idn_sb[:])
        for q in range(wch):
            col = nl * wch + q                 # sample-major columns
            zcol = zT[:, q * n_loc + nl:q * n_loc + nl + 1]
            sg = scrp.tile([128, w], F32, tag="sg", name=f"sg{nl}_{q}")
            se = scrp.tile([128, w], F32, tag="se", name=f"se{nl}_{q}")
            # rank0 = #{j: z[j] > z[i]}
            nc.vector.scalar_tensor_tensor(
                out=sg[:], in0=zrow[:], scalar=zcol, in1=ones_bc,
                op0=ALU.is_gt, op1=ALU.mult,
                accum_out=rank0[:, col:col + 1])
            # rankt = #{j < i: z[j] == z[i]}  (tie break by index)
            nc.vector.scalar_tensor_tensor(
                out=se[:], in0=zrow[:], scalar=zcol,
                in1=trim_sb[:, q * w:(q + 1) * w],
                op0=ALU.is_equal, op1=ALU.mult,
                accum_out=rankt[:, col:col + 1])
            nc.vector.tensor_add(rankf[:, col:col + 1], rank0[:, col:col + 1],
                                 rankt[:, col:col + 1])
            nc.vector.tensor_scalar(keep[:, col:col + 1], rankf[:, col:col + 1],
                                    float(out_w), None, op0=ALU.is_lt)

        # within-chunk prefix counts + per-chunk totals, one matmul each
        keep_s = keep[:, nl * wch:(nl + 1) * wch]  # [128, wch]
        within_ps = psum_small.tile([128, wch], F32, tag="mm", name=f"wi{nl}")
        nc.tensor.matmul(out=within_ps[:], lhsT=triu_sb[:], rhs=keep_s,
                         start=True, stop=True)
        cnt_ps = psum_small.tile([128, wch], F32, tag="mm", name=f"cn{nl}")
        nc.tensor.matmul(out=cnt_ps[:], lhsT=ones_sb[:], rhs=keep_s,
                         start=True, stop=True)
        pos = mainp.tile([128, wch], F32, name=f"pos{nl}", tag=f"pos{nl}")
        nc.vector.tensor_copy(pos[:, 0:1], within_ps[:, 0:1])
        off = mainp.tile([128, wch], F32, name=f"off{nl}", tag=f"off{nl}")
        nc.vector.tensor_copy(off[:, 0:1], cnt_ps[:, 0:1])
        for m in range(1, wch):
            nc.vector.tensor_add(pos[:, m:m + 1], within_ps[:, m:m + 1],
                                 off[:, m - 1:m])
            if m < wch - 1:
                nc.vector.tensor_add(off[:, m:m + 1], cnt_ps[:, m:m + 1],
                                     off[:, m - 1:m])

        # one-hot [i, slot];  idx[slot] = sum_i onehot * i  (+ nl*w)
        idx_ps = psum_small.tile([out_w, 1], F32, tag="mm", name=f"idx{nl}")
        for q in range(wch):
            o = ohp.tile([128, out_w], F32, tag="oh", name=f"oh{nl}_{q}")
            nc.vector.tensor_scalar(o[:], irow_sb[:], pos[:, q:q + 1],
                                    keep[:, nl * wch + q:nl * wch + q + 1],
                                    op0=ALU.is_equal, op1=ALU.mult)
            nc.tensor.matmul(out=idx_ps[:], lhsT=o[:], rhs=icol_sb[:, q:q + 1],
                             start=(q == 0), stop=(q == wch - 1))
        idx_f = mainp.tile([out_w, 1], F32, name=f"idxf{nl}", tag=f"idxf{nl}")
        nc.vector.tensor_single_scalar(idx_f[:], idx_ps[:], float(nl * w),
                                       ALU.add)
        idx_i = mainp.tile([out_w, 1], I32, name=f"idxi{nl}", tag=f"idxi{nl}")
        nc.vector.tensor_copy(idx_i[:], idx_f[:])

        # ---------------- phase 3: row gather ---------------------------
        # each index fetches a contiguous gw*h slice of the (c, h) row,
        # selected by element_offset; output writes alternate HWDGE rings.
        for q in range(c // gw):
            gt = gp.tile([out_w, gw * h], F32, tag="gt", name=f"gt{nl}_{q}")
            nc.gpsimd.indirect_dma_start(
                out=gt[:], out_offset=None, in_=x_rows,
                in_offset=IndirectOffsetOnAxis(ap=idx_i[:], axis=0),
                element_offset=q * gw * h)
            dst = out_ap[nl, q * gw:(q + 1) * gw].rearrange("i j h -> j i h")
            hw_engs[q % 2].dma_start(
                out=dst, in_=gt[:].rearrange("j (i h) -> j i h", i=gw))

    gp.release()
    ohp.release()
    scrp.release()
    psum_z.release()
    psum_small.release()
    dramp.release()
    mainp.release()
    constp.release()
    return post_hooks


def host_inputs(w1, b1, gamma, beta, w2, b2, cfg: Cfg):
    """Shared (non-sharded) input tensors, prepacked for the kernel."""
    c, w, red, out_w, wch = cfg.c, cfg.w, cfg.red, cfg.out_w, cfg.wch
    f = np.float32
    # w1.T columns duplicated: [128, wch * 2*red], chunk k holds
    # [w1.T chunk | w1.T chunk] so the matmul emits h twice.
    w1t = w1.T.reshape(wch, 128, red)
    w1t2 = np.concatenate([w1t, w1t], axis=2)            # [wch, 128, 2*red]
    w1t2 = np.ascontiguousarray(
        w1t2.transpose(1, 0, 2).reshape(128, wch * 2 * red)).astype(f)
    b1d = np.concatenate([b1, b1]).reshape(2 * red, 1)
    w2t = np.ascontiguousarray(w2.T).astype(f)
    b2t = np.ascontiguousarray(b2.reshape(wch, 128).T).astype(f)
    irow = np.tile(np.arange(out_w, dtype=f), (128, 1))
    icol = (np.arange(wch, dtype=f)[None, :] * 128
            + np.arange(128, dtype=f)[:, None])
    jj = np.arange(w, dtype=np.int64)[None, None, :]
    ii = (np.arange(wch, dtype=np.int64)[:, None, None] * 128
          + np.arange(128, dtype=np.int64)[None, :, None])
    trim = (jj < ii).astype(f).transpose(1, 0, 2).reshape(128, wch * w)
    mskhi = (np.arange(128) >= red).astype(f).reshape(128, 1)
    return {
        "w1t2": w1t2,
        "w2t": w2t,
        "b1d": np.ascontiguousarray(b1d).astype(f),
        "gmc": np.ascontiguousarray(gamma.reshape(red, 1)).astype(f),
        "btc": np.ascontiguousarray(beta.reshape(red, 1)).astype(f),
        "b2t": b2t,
        "idn": np.eye(128, dtype=f),
        "ones": np.ones((128, 128), dtype=f),
        "triu": np.triu(np.ones((128, 128), dtype=f), k=1),
        "irow": irow,
        "icol": np.ascontiguousarray(icol),
        "trim": np.ascontiguousarray(trim),
        "mskhi": mskhi,
        "msklo": np.ascontiguousarray(1.0 - mskhi).astype(f),
        "epsc": np.full((128, 1), BN_EPS, dtype=f),
    }


def build_nc(cfg: Cfg):
    nc = bacc.Bacc("TRN2", target_bir_lowering=False, debug=False,
                   num_devices=cfg.ncores)
    n_loc, c, w, h, red, out_w, wch = (cfg.n_loc, cfg.c, cfg.w, cfg.h,
                                       cfg.red, cfg.out_w, cfg.wch)
    ins = {}
    ins["xs"] = nc.dram_tensor("xs", [n_loc, w, c, h], F32,
                               kind="ExternalInput").ap()
    for name, shape in [
        ("w1t2", [128, wch * 2 * red]),
        ("w2t", [red, w]),
        ("b1d", [2 * red, 1]),
        ("gmc", [red, 1]),
        ("btc", [red, 1]),
        ("b2t", [128, wch]),
        ("idn", [128, 128]),
        ("ones", [128, 128]),
        ("triu", [128, 128]),
        ("irow", [128, out_w]),
        ("icol", [128, wch]),
        ("trim", [128, wch * w]),
        ("mskhi", [128, 1]),
        ("msklo", [128, 1]),
        ("epsc", [128, 1]),
    ]:
        ins[name] = nc.dram_tensor(name, shape, F32, kind="ExternalInput").ap()
    out = nc.dram_tensor("out", [n_loc, c, out_w, h], F32,
                         kind="ExternalOutput").ap()
    with tile.TileContext(nc) as tc:
        post_hooks = kernel_body(tc, out, ins, cfg)
    for guard, sem, thresh, consumers in post_hooks:
        guard.wait_op(sem, thresh, "sem-ge")
        # the guard NOP must precede its consumers in the final program
        # order, else the wait guards nothing
        pos = {}
        i = 0
        for blk in nc.m.functions[0].blocks:
            for x in blk.instructions:
                pos[x.name] = i
                i += 1
        gi = pos[guard.ins.name]
        for cons in consumers:
            if pos[cons.ins.name] < gi:
                raise RuntimeError(
                    f"rdma guard {guard.ins.name} scheduled after consumer "
                    f"{cons.ins.name}")
    nc.compile()
    return nc


_CACHE = {}


def get_nc(cfg=None):
    cfg = cfg or Cfg()
    if "nc" not in _CACHE:
        _CACHE["nc"] = build_nc(cfg)
    return _CACHE["nc"]


def make_in_maps(inputs, cfg=None):
    cfg = cfg or Cfg()
    x = np.ascontiguousarray(np.asarray(inputs["x"], dtype=np.float32))
    shared = host_inputs(np.asarray(inputs["w1"]), np.asarray(inputs["b1"]),
                         np.asarray(inputs["gamma"]),
                         np.asarray(inputs["beta"]),
                         np.asarray(inputs["w2"]), np.asarray(inputs["b2"]),
                         cfg)
    in_maps = []
    for i in range(cfg.ncores):
        m = dict(shared)
        m["xs"] = np.ascontiguousarray(
            x[i * cfg.n_loc:(i + 1) * cfg.n_loc].transpose(0, 2, 1, 3))
        in_maps.append(m)
    return in_maps


def kernel(**inputs):
    cfg = Cfg()
    nc = get_nc(cfg)
    in_maps = make_in_maps(inputs, cfg)
    res = run_bass_kernel_spmd(nc, in_maps, list(range(cfg.ncores)))
    return np.concatenate([r["out"] for r in res.results], axis=0)


# revision 31
# speedup vs baseline: 1.0860x; 1.0860x over previous
"""Trainium2 Bass kernel for nn_AttentionPool_v1 (topk_masking).

Reference computation (N=16, C=64, W=512, H=512, RED=64, OUT_W=128):
    pooled = max(x, axes=(C, H))                  # [N, W]
    h      = pooled @ w1.T + b1                   # [N, RED]
    h      = BN1d(h, batch stats) -> relu         # [N, RED]
    att    = softmax(h @ w2.T + b2, axis=1)       # [N, W]
    idx    = sort(top_k(att, OUT_W).indices)      # [N, OUT_W]
    out    = x[n, :, idx[n], :]                   # [N, C, OUT_W, H]

Sharding: data-parallel over batch N across 8 cores (2 samples/core).
BatchNorm batch statistics are made exact via ONE tiny AllReduce of
[sum(h); sum(h^2)] packed into a [128, 1] payload (h is duplicated into
partitions 64..127 by loading w1/b1 twice), with var = E[h^2] - mu^2.

Device algorithm notes:
  * softmax is monotonic per-row => top-k on the logits z directly.
  * top-k via ranking: rank[i] = #{j: z[j] > z[i]} + #{j<i: z[j]==z[i]};
    keep = rank < OUT_W selects exactly the top-k set with jax.lax.top_k
    tie-breaking; output order is ascending index == jnp.sort(idx).
    The two rank sums are computed with fused scalar_tensor_tensor ops
    (compare + mask-multiply + accumulate in one instruction), reading
    the replicated z row straight out of PSUM.
  * compaction: one triu matmul gives within-chunk prefix sums, one
    ones matmul gives per-chunk totals; one-hot(pos) matmul iota gives
    the sorted idx values; indirect DMA row-gather moves the rows.
  * x is fed host-transposed as [n, w, c, h] so the (C,H) max-pool
    reads contiguous per-partition runs and the gather fetches
    contiguous (c, h) row slices with a single [128, 1] index column
    (multi-column indirect offsets hang the hardware DGE).
  * phase-1 x loads alternate between the two HWDGE rings (sync/scalar
    engines); constants load via the SWDGE ring to keep the x rings
    clean; phase-3 output writes also alternate the HWDGE rings.
"""

import numpy as np

import concourse.bacc as bacc
import concourse.bass as bass
import concourse.mybir as mybir
import concourse.tile as tile
from concourse.bass import IndirectOffsetOnAxis
from concourse.bass_utils import run_bass_kernel_spmd

F32 = mybir.dt.float32
I32 = mybir.dt.int32
ALU = mybir.AluOpType
AX = mybir.AxisListType
ACTF = mybir.ActivationFunctionType

# full-problem config
N_FULL, C_FULL, W_FULL, H_FULL = 16, 64, 512, 512
RED_FULL, OUTW_FULL = 64, 128
NCORES_FULL = 8
BN_EPS = 1e-5


class Cfg:
    def __init__(self, ncores=NCORES_FULL, n_loc=N_FULL // NCORES_FULL, c=C_FULL,
                 w=W_FULL, h=H_FULL, red=RED_FULL, out_w=OUTW_FULL,
                 chains=8, gw=8, cpt=8, gbufs=8, use_rdma=False):
        self.use_rdma = use_rdma          # remote-DMA batch-stats exchange
        assert w % 128 == 0
        self.ncores, self.n_loc, self.c, self.w, self.h = ncores, n_loc, c, w, h
        self.red, self.out_w = red, out_w
        self.wch = w // 128
        self.chains = chains              # x staging buffers (phase 1)
        self.gw = gw                      # channels per indirect gather
        self.gbufs = gbufs                # gather staging buffers (phase 3)
        self.cpt = min(cpt, c)            # channels per phase-1 tile
        assert c % self.cpt == 0 and c % gw == 0
        assert out_w <= 128 and red <= 128 and 2 * red <= 128


def kernel_body(tc, out_ap, ins, cfg: Cfg):
    """Emit the kernel IR. `ins` is a dict name -> DRAM AP (see host_inputs).

    Returns a list of (instruction, semaphore, threshold) waits to attach
    AFTER tile scheduling (cross-core arrivals the scheduler cannot model).
    """
    nc = tc.nc
    post_hooks = []
    n_loc, c, w, h, red, out_w = (cfg.n_loc, cfg.c, cfg.w, cfg.h, cfg.red,
                                  cfg.out_w)
    wch, gw = cfg.wch, cfg.gw
    xs = ins["xs"]
    group = [list(range(cfg.ncores))]
    nbatch = float(cfg.ncores * n_loc)

    # constants staged via the SWDGE ring (keeps HWDGE rings free for x)
    constp = tc.alloc_tile_pool(name="const", bufs=1)

    def cload(shape, src, name):
        t = constp.tile(shape, F32, name=name)
        nc.gpsimd.dma_start(t[:], ins[src])
        return t

    w1t_sb = cload([128, wch * 2 * red], "w1t2", "w1t")   # duplicated columns
    w2t_sb = cload([red, w], "w2t", "w2t")
    b1_sb = cload([128, 1], "b1d", "b1d")                 # duplicated rows
    gm_sb = cload([red, 1], "gmc", "gmc")
    bt_sb = cload([red, 1], "btc", "btc")
    b2_sb = cload([128, wch], "b2t", "b2t")
    idn_sb = cload([128, 128], "idn", "idn")
    ones_sb = cload([128, 128], "ones", "ones")
    triu_sb = cload([128, 128], "triu", "triu")
    irow_sb = cload([128, out_w], "irow", "irow")
    icol_sb = cload([128, wch], "icol", "icol")
    trim_sb = cload([128, wch * w], "trim", "trim")
    msk_sb = cload([128, 1], "mskhi", "mskhi")            # 1 on partitions>=red
    inv_sb = cload([128, 1], "msklo", "msklo")            # 1 on partitions<red
    eps_sb = cload([128, 1], "epsc", "epsc")              # BN_EPS everywhere

    # touch the Sqrt activation early: the first ACTIVATE on a fresh table
    # costs a ~1.3us ACT_TABLE_LOAD, which otherwise lands on the
    # post-collective critical path
    warmp = tc.alloc_tile_pool(name="warm", bufs=1)
    sq_warm = warmp.tile([64, 1], F32)
    nc.scalar.sqrt(sq_warm[:], eps_sb[0:64, :])
    warmp.release()

    mainp = tc.alloc_tile_pool(name="main", bufs=1)
    dramp = tc.alloc_tile_pool(name="dram", bufs=1, space="DRAM")
    psum_small = tc.alloc_tile_pool(name="ps_small", bufs=2, space="PSUM")
    psum_z = tc.alloc_tile_pool(name="ps_z", bufs=2, space="PSUM")

    hw_engs = [nc.sync, nc.scalar]        # the two HWDGE rings

    if cfg.use_rdma:
        # Batch-stats exchange via direct cross-core SBUF writes instead of
        # the ncfw collective.  Core r sends its [s1|s2] column pair to core
        # (r XOR d) landing at column pair 2d; sums over the landing tile are
        # permutation-invariant, so no rank-dependent addressing is needed.
        # Descriptors are prepared here (SWDGE ring is idle during phase 1)
        # and fired by one trigger_dma once the stats are written.
        stats2 = mainp.tile([128, 2], F32, name="stats2")
        land = mainp.tile([128, 2 * cfg.ncores], F32, name="land")
        nc.gpsimd.memset(stats2[:], 0.0)
        rd_rsem = nc.alloc_semaphore("rdma_stats_rsem")
        rd_lsem = nc.alloc_semaphore("rdma_stats_lsem")
        rd_ssem = nc.alloc_semaphore("rdma_stats_ssem")
        for dist in range(1, cfg.ncores):
            rdests = [None] * cfg.ncores
            rdests[dist] = (0, dist)
            nc.gpsimd.remote_dma_broadcast(
                out_ap=land[:, 2 * dist:2 * dist + 2],
                in_ap=stats2[:, 0:2],
                remote_sem=rd_rsem, local_sem=rd_lsem, rdests=rdests)

    # ---------------- phase 1: pooled[w] = max over (c, h) --------------
    # xs layout is [n, w, c, h] (host-transposed), so each w-partition
    # reads a contiguous (c, h) run.
    cpt = cfg.cpt                     # channels per tile
    nct = c // cpt
    pooledT = mainp.tile([128, wch * n_loc], F32)  # [p, k*n_loc+n]
    with tc.tile_pool(name="xp", bufs=cfg.chains) as xp, \
            tc.tile_pool(name="cmp", bufs=1) as cm_pool:
        colmax = []
        for n in range(n_loc):
            cm = cm_pool.tile([128, wch * nct], F32, name=f"colmax{n}",
                              tag=f"colmax{n}")
            colmax.append(cm)
        di = 0
        for n in range(n_loc):
            for k in range(wch):
                for t in range(nct):
                    xt = xp.tile([128, cpt * h], F32, tag="xt",
                                 name=f"xt{n}_{k}_{t}")
                    src = xs[n, k * 128:(k + 1) * 128, t * cpt:(t + 1) * cpt, :]
                    hw_engs[di % 2].dma_start(
                        out=xt[:].rearrange("p (cc hh) -> p cc hh", cc=cpt),
                        in_=src)
                    di += 1
                    nc.vector.reduce_max(
                        out=colmax[n][:, k * nct + t: k * nct + t + 1],
                        in_=xt[:], axis=AX.X)
        for n in range(n_loc):
            for k in range(wch):
                nc.vector.reduce_max(
                    out=pooledT[:, k * n_loc + n: k * n_loc + n + 1],
                    in_=colmax[n][:, k * nct:(k + 1) * nct], axis=AX.X)

    # ---------------- phase 2: MLP + BN + ranking -----------------------
    # hT2[p, nl]: partitions 0..red-1 and red..2*red-1 both hold h (the
    # duplicated w1/b1 make the matmul emit two copies), so one [128, 1]
    # reduction carries both sum(h) and sum(h^2) into a single AllReduce.
    hT_ps = psum_small.tile([2 * red, n_loc], F32, tag="mm")
    for k in range(wch):
        nc.tensor.matmul(out=hT_ps[:],
                         lhsT=w1t_sb[:, k * 2 * red:(k + 1) * 2 * red],
                         rhs=pooledT[:, k * n_loc:(k + 1) * n_loc],
                         start=(k == 0), stop=(k == wch - 1))
    hT2 = mainp.tile([2 * red, n_loc], F32)
    nc.vector.tensor_scalar_add(hT2[:], hT_ps[:], b1_sb[:, :1])

    if cfg.use_rdma:
        # stats2 = [sum_n h | sum_n h^2], both on partitions 0..red-1
        nc.vector.reduce_sum(out=stats2[0:red, 0:1], in_=hT2[0:red, :],
                             axis=AX.X)
        sq_scr = mainp.tile([red, n_loc], F32, name="sq_scr")
        nc.vector.scalar_tensor_tensor(
            out=sq_scr[:], in0=hT2[0:red, :], scalar=1.0, in1=hT2[0:red, :],
            op0=ALU.mult, op1=ALU.mult, accum_out=stats2[0:red, 1:2])
        nc.vector.tensor_copy(land[:, 0:2], stats2[:, 0:2])
        nc.gpsimd.trigger_dma(count=None).then_inc(rd_ssem, 1)
        # Arrival guard: waits on rd_ssem, incremented by the trigger
        # (which the scheduler's sim models and which transitively depends
        # on the stats being written), so the scheduler anchors the guard
        # late instead of hoisting a dep-less wait to the queue front.
        # The wait for the 7 peers' arrivals is attached post-scheduling
        # (the single-core sim cannot model remote semaphore increments);
        # build_nc asserts the guard still precedes the reduces in the
        # final program order.
        guard = nc.vector.wait_ge(rd_ssem, 1)
        land_v = land[:].rearrange("p (d s) -> p s d", s=2)
        allsum = mainp.tile([128, 2], F32, name="allsum")
        red1 = nc.vector.reduce_sum(out=allsum[:, 0:1], in_=land_v[:, 0:1, :],
                                    axis=AX.X)
        red2 = nc.vector.reduce_sum(out=allsum[:, 1:2], in_=land_v[:, 1:2, :],
                                    axis=AX.X)
        post_hooks.append((guard, rd_rsem, 2 * (cfg.ncores - 1), [red1, red2]))
        musum = allsum[0:red, 0:1]
        s2sum = allsum[0:red, 1:2]
    else:
        # g = h*mskhi + msklo  (1 on lower half, h on upper half)
        gsel = mainp.tile([2 * red, n_loc], F32)
        nc.vector.scalar_tensor_tensor(
            out=gsel[:], in0=hT2[:], scalar=msk_sb[:, :1],
            in1=inv_sb[:, :1].to_broadcast([2 * red, n_loc]),
            op0=ALU.mult, op1=ALU.add)
        # stats[p] = sum_n h*g  ->  lower half sum(h), upper half sum(h^2)
        scr_st = mainp.tile([2 * red, n_loc], F32)
        stats = mainp.tile([2 * red, 1], F32)
        nc.vector.scalar_tensor_tensor(
            out=scr_st[:], in0=hT2[:], scalar=1.0, in1=gsel[:],
            op0=ALU.mult, op1=ALU.mult, accum_out=stats[:])

        # two concurrent 64-element AllReduces (a single 128-element payload
        # trips a ~7x slower ncfw path); both inputs are ready together since
        # the variance is uncentered.  SWDGE staging keeps the input DMA off
        # the busy HWDGE completion-semaphore lanes.
        cc1_i = dramp.tile([red, 1], F32)
        cc1_o = dramp.tile([red, 1], F32)
        cc2_i = dramp.tile([red, 1], F32)
        cc2_o = dramp.tile([red, 1], F32)
        nc.gpsimd.dma_start(cc1_i[:], stats[0:red, :])
        nc.gpsimd.dma_start(cc2_i[:], stats[red:2 * red, :])
        nc.gpsimd.collective_compute("AllReduce", ALU.add,
                                     replica_groups=group,
                                     ins=[cc1_i.opt()], outs=[cc1_o.opt()])
        nc.gpsimd.collective_compute("AllReduce", ALU.add,
                                     replica_groups=group,
                                     ins=[cc2_i.opt()], outs=[cc2_o.opt()])
        musum = mainp.tile([red, 1], F32)
        s2sum = mainp.tile([red, 1], F32)
        nc.sync.dma_start(musum[:], cc1_o[:])
        nc.scalar.dma_start(s2sum[:], cc2_o[:])

    # rstd = 1/sqrt(s2sum/nb - mu^2 + eps);  hr = relu(gamma*rstd*(h-mu)+beta)
    mu = mainp.tile([red, 1], F32)
    nc.vector.tensor_scalar_mul(mu[:], musum[:], 1.0 / nbatch)
    nmu = mainp.tile([red, 1], F32)
    nc.scalar.mul(nmu[:], mu[:], -1.0)
    emm = mainp.tile([red, 1], F32)                      # eps - mu^2
    nc.vector.scalar_tensor_tensor(
        out=emm[:], in0=mu[:], scalar=nmu[:, :1],
        in1=eps_sb[0:red, :1].to_broadcast([red, 1]),
        op0=ALU.mult, op1=ALU.add)
    sd = mainp.tile([red, 1], F32)
    nc.scalar.activation(sd[:], s2sum[:], ACTF.Sqrt,
                         bias=emm[:, :1], scale=1.0 / nbatch)
    rstd = mainp.tile([red, 1], F32)
    nc.vector.reciprocal(rstd[:], sd[:])
    gs = mainp.tile([red, 1], F32)
    nc.vector.tensor_mul(gs[:], gm_sb[:], rstd[:])
    centered = mainp.tile([red, n_loc], F32)
    nc.vector.tensor_scalar_sub(centered[:], hT2[0:red, :], mu[:, :1])
    hr = mainp.tile([red, n_loc], F32)
    nc.vector.scalar_tensor_tensor(
        out=hr[:], in0=centered[:], scalar=gs[:, :1],
        in1=bt_sb[:, :1].to_broadcast([red, n_loc]),
        op0=ALU.mult, op1=ALU.add)
    nc.vector.tensor_scalar_max(hr[:], hr[:], 0.0)

    # zT[wq, nl] = w2 @ hr + b2   (logits, transposed)
    zT = mainp.tile([128, wch * n_loc], F32)  # col = q*n_loc + nl
    for q in range(wch):
        z_ps = psum_small.tile([128, n_loc], F32, tag="mm")
        nc.tensor.matmul(out=z_ps[:], lhsT=w2t_sb[:, q * 128:(q + 1) * 128],
                         rhs=hr[:], start=True, stop=True)
        nc.vector.tensor_scalar_add(zT[:, q * n_loc:(q + 1) * n_loc], z_ps[:],
                                    b2_sb[:, q:q + 1])

    # per-sample: ranking -> keep -> pos -> idx -> row gather.  Sample 0's
    # gather DMAs start while sample 1's ranking still runs.
    rank0 = mainp.tile([128, wch * n_loc], F32)
    rankt = mainp.tile([128, wch * n_loc], F32)
    rankf = mainp.tile([128, wch * n_loc], F32)
    keep = mainp.tile([128, wch * n_loc], F32)
    x_rows = xs.rearrange("n w c h -> (n w) (c h)")
    scrp = tc.alloc_tile_pool(name="scr", bufs=2 * wch)
    ohp = tc.alloc_tile_pool(name="ohp", bufs=wch * n_loc)
    gp = tc.alloc_tile_pool(name="gp", bufs=cfg.gbufs)
    ones_bc = ones_sb[:, 0:1].to_broadcast([128, w])

    for nl in range(n_loc):
        # replicated logits row, straight in PSUM: zrow[p, j] = z[j]
        zrow = psum_z.tile([128, w], F32, tag="zrow", name=f"zrow{nl}")
        for q in range(wch):
            col = q * n_loc + nl
            nc.tensor.transpose(
                out=zrow[:, q * 128:(q + 1) * 128],
                in_=zT[:, col:col + 1].to_broadcast([128, 128]),
                identity=idn_sb[:])
        for q in range(wch):
            col = nl * wch + q                 # sample-major columns
            zcol = zT[:, q * n_loc + nl:q * n_loc + nl + 1]
            sg = scrp.tile([128, w], F32, tag="sg", name=f"sg{nl}_{q}")
            se = scrp.tile([128, w], F32, tag="se", name=f"se{nl}_{q}")
            # rank0 = #{j: z[j] > z[i]}
            nc.vector.scalar_tensor_tensor(
                out=sg[:], in0=zrow[:], scalar=zcol, in1=ones_bc,
                op0=ALU.is_gt, op1=ALU.mult,
                accum_out=rank0[:, col:col + 1])
            # rankt = #{j < i: z[j] == z[i]}  (tie break by index)
            nc.vector.scalar_tensor_tensor(
                out=se[:], in0=zrow[:], scalar=zcol,
                in1=trim_sb[:, q * w:(q + 1) * w],
                op0=ALU.is_equal, op1=ALU.mult,
                accum_out=rankt[:, col:col + 1])
            nc.vector.tensor_add(rankf[:, col:col + 1], rank0[:, col:col + 1],
                                 rankt[:, col:col + 1])
            nc.vector.tensor_scalar(keep[:, col:col + 1], rankf[:, col:col + 1],
                                    float(out_w), None, op0=ALU.is_lt)

        # within-chunk prefix counts + per-chunk totals, one matmul each
        keep_s = keep[:, nl * wch:(nl + 1) * wch]  # [128, wch]
        within_ps = psum_small.tile([128, wch], F32, tag="mm", name=f"wi{nl}")
        nc.tensor.matmul(out=within_ps[:], lhsT=triu_sb[:], rhs=keep_s,
                         start=True, stop=True)
        cnt_ps = psum_small.tile([128, wch], F32, tag="mm", name=f"cn{nl}")
        nc.tensor.matmul(out=cnt_ps[:], lhsT=ones_sb[:], rhs=keep_s,
                         start=True, stop=True)
        pos = mainp.tile([128, wch], F32, name=f"pos{nl}", tag=f"pos{nl}")
        nc.vector.tensor_copy(pos[:, 0:1], within_ps[:, 0:1])
        off = mainp.tile([128, wch], F32, name=f"off{nl}", tag=f"off{nl}")
        nc.vector.tensor_copy(off[:, 0:1], cnt_ps[:, 0:1])
        for m in range(1, wch):
            nc.vector.tensor_add(pos[:, m:m + 1], within_ps[:, m:m + 1],
                                 off[:, m - 1:m])
            if m < wch - 1:
                nc.vector.tensor_add(off[:, m:m + 1], cnt_ps[:, m:m + 1],
                                     off[:, m - 1:m])

        # one-hot [i, slot];  idx[slot] = sum_i onehot * i  (+ nl*w)
        idx_ps = psum_small.tile([out_w, 1], F32, tag="mm", name=f"idx{nl}")
        for q in range(wch):
            o = ohp.tile([128, out_w], F32, tag="oh", name=f"oh{nl}_{q}")
            nc.vector.tensor_scalar(o[:], irow_sb[:], pos[:, q:q + 1],
                                    keep[:, nl * wch + q:nl * wch + q + 1],
                                    op0=ALU.is_equal, op1=ALU.mult)
            nc.tensor.matmul(out=idx_ps[:], lhsT=o[:], rhs=icol_sb[:, q:q + 1],
                             start=(q == 0), stop=(q == wch - 1))
        idx_f = mainp.tile([out_w, 1], F32, name=f"idxf{nl}", tag=f"idxf{nl}")
        nc.vector.tensor_single_scalar(idx_f[:], idx_ps[:], float(nl * w),
                                       ALU.add)
        idx_i = mainp.tile([out_w, 1], I32, name=f"idxi{nl}", tag=f"idxi{nl}")
        nc.vector.tensor_copy(idx_i[:], idx_f[:])

        # ---------------- phase 3: row gather ---------------------------
        # each index fetches a contiguous gw*h slice of the (c, h) row,
        # selected by element_offset; output writes alternate HWDGE rings.
        for q in range(c // gw):
            gt = gp.tile([out_w, gw * h], F32, tag="gt", name=f"gt{nl}_{q}")
            nc.gpsimd.indirect_dma_start(
                out=gt[:], out_offset=None, in_=x_rows,
                in_offset=IndirectOffsetOnAxis(ap=idx_i[:], axis=0),
                element_offset=q * gw * h)
            dst = out_ap[nl, q * gw:(q + 1) * gw].rearrange("i j h -> j i h")
            hw_engs[q % 2].dma_start(
                out=dst, in_=gt[:].rearrange("j (i h) -> j i h", i=gw))

    gp.release()
    ohp.release()
    scrp.release()
    psum_z.release()
    psum_small.release()
    dramp.release()
    mainp.release()
    constp.release()
    return post_hooks


def host_inputs(w1, b1, gamma, beta, w2, b2, cfg: Cfg):
    """Shared (non-sharded) input tensors, prepacked for the kernel."""
    c, w, red, out_w, wch = cfg.c, cfg.w, cfg.red, cfg.out_w, cfg.wch
    f = np.float32
    # w1.T columns duplicated: [128, wch * 2*red], chunk k holds
    # [w1.T chunk | w1.T chunk] so the matmul emits h twice.
    w1t = w1.T.reshape(wch, 128, red)
    w1t2 = np.concatenate([w1t, w1t], axis=2)            # [wch, 128, 2*red]
    w1t2 = np.ascontiguousarray(
        w1t2.transpose(1, 0, 2).reshape(128, wch * 2 * red)).astype(f)
    b1d = np.concatenate([b1, b1]).reshape(2 * red, 1)
    w2t = np.ascontiguousarray(w2.T).astype(f)
    b2t = np.ascontiguousarray(b2.reshape(wch, 128).T).astype(f)
    irow = np.tile(np.arange(out_w, dtype=f), (128, 1))
    icol = (np.arange(wch, dtype=f)[None, :] * 128
            + np.arange(128, dtype=f)[:, None])
    jj = np.arange(w, dtype=np.int64)[None, None, :]
    ii = (np.arange(wch, dtype=np.int64)[:, None, None] * 128
          + np.arange(128, dtype=np.int64)[None, :, None])
    trim = (jj < ii).astype(f).transpose(1, 0, 2).reshape(128, wch * w)
    mskhi = (np.arange(128) >= red).astype(f).reshape(128, 1)
    return {
        "w1t2": w1t2,
        "w2t": w2t,
        "b1d": np.ascontiguousarray(b1d).astype(f),
        "gmc": np.ascontiguousarray(gamma.reshape(red, 1)).astype(f),
        "btc": np.ascontiguousarray(beta.reshape(red, 1)).astype(f),
        "b2t": b2t,
        "idn": np.eye(128, dtype=f),
        "ones": np.ones((128, 128), dtype=f),
        "triu": np.triu(np.ones((128, 128), dtype=f), k=1),
        "irow": irow,
        "icol": np.ascontiguousarray(icol),
        "trim": np.ascontiguousarray(trim),
        "mskhi": mskhi,
        "msklo": np.ascontiguousarray(1.0 - mskhi).astype(f),
        "epsc": np.full((128, 1), BN_EPS, dtype=f),
    }


def build_nc(cfg: Cfg):
    nc = bacc.Bacc("TRN2", target_bir_lowering=False, debug=False,
                   num_devices=cfg.ncores)
    n_loc, c, w, h, red, out_w, wch = (cfg.n_loc, cfg.c, cfg.w, cfg.h,
                                       cfg.red, cfg.out_w, cfg.wch)
    ins = {}
    ins["xs"] = nc.dram_tensor("xs", [n_loc, w, c, h], F32,
                               kind="ExternalInput").ap()
    for name, shape in [
        ("w1t2", [128, wch * 2 * red]),
        ("w2t", [red, w]),
        ("b1d", [2 * red, 1]),
        ("gmc", [red, 1]),
        ("btc", [red, 1]),
        ("b2t", [128, wch]),
        ("idn", [128, 128]),
        ("ones", [128, 128]),
        ("triu", [128, 128]),
        ("irow", [128, out_w]),
        ("icol", [128, wch]),
        ("trim", [128, wch * w]),
        ("mskhi", [128, 1]),
        ("msklo", [128, 1]),
        ("epsc", [128, 1]),
    ]:
        ins[name] = nc.dram_tensor(name, shape, F32, kind="ExternalInput").ap()
    out = nc.dram_tensor("out", [n_loc, c, out_w, h], F32,
                         kind="ExternalOutput").ap()
    with tile.TileContext(nc) as tc:
        post_hooks = kernel_body(tc, out, ins, cfg)
    for guard, sem, thresh, consumers in post_hooks:
        guard.wait_op(sem, thresh, "sem-ge")
        # the guard NOP must precede its consumers in the final program
        # order, else the wait guards nothing
        pos = {}
        i = 0
        for blk in nc.m.functions[0].blocks:
            for x in blk.instructions:
                pos[x.name] = i
                i += 1
        gi = pos[guard.ins.name]
        for cons in consumers:
            if pos[cons.ins.name] < gi:
                raise RuntimeError(
                    f"rdma guard {guard.ins.name} scheduled after consumer "
                    f"{cons.ins.name}")
    nc.compile()
    return nc


_CACHE = {}


def get_nc(cfg=None):
    cfg = cfg or Cfg()
    if "nc" not in _CACHE:
        _CACHE["nc"] = build_nc(cfg)
    return _CACHE["nc"]


def make_in_maps(inputs, cfg=None):
    cfg = cfg or Cfg()
    x = np.ascontiguousarray(np.asarray(inputs["x"], dtype=np.float32))
    shared = host_inputs(np.asarray(inputs["w1"]), np.asarray(inputs["b1"]),
                         np.asarray(inputs["gamma"]),
                         np.asarray(inputs["beta"]),
                         np.asarray(inputs["w2"]), np.asarray(inputs["b2"]),
                         cfg)
    in_maps = []
    for i in range(cfg.ncores):
        m = dict(shared)
        m["xs"] = np.ascontiguousarray(
            x[i * cfg.n_loc:(i + 1) * cfg.n_loc].transpose(0, 2, 1, 3))
        in_maps.append(m)
    return in_maps


def kernel(**inputs):
    cfg = Cfg()
    nc = get_nc(cfg)
    in_maps = make_in_maps(inputs, cfg)
    res = run_bass_kernel_spmd(nc, in_maps, list(range(cfg.ncores)))
    return np.concatenate([r["out"] for r in res.results], axis=0)
